# revision 1
# baseline (speedup 1.0000x reference)
"""CODA-Prompt forward kernel for 8 TRN2 NeuronCores (data-parallel over batch).

Reference computation (forward only; stop_gradient is identity):
    K = (task_count + 1) * 10            # active pool slice, all branches
    x_mean[b,d]  = mean_n x[b,n,d]
    aq[b,k]      = (x_mean . (att[k]*nK[k])) / max(||x_mean*att[k]||, eps)
    P_[b,l,d]    = sum_k aq[b,k] * prompt[k,l,d]
    out          = concat([P_, x], axis=1)            # [B, 8+197, 768]

Device kernel per core (B=32 of 256 batches), built for DMA efficiency:
  x arrives flat+padded [B*197+1, 768].  Each batch is one fully
  sequential DMA in token-pair layout [99, 2, 768] (6 KB runs), copied
  back out to out_flat rows [205b+8, 205b+206) (also sequential).  The
  199th row of each tile is the next batch's token 0 (garbage); its
  out-write lands on the P_ row of b+1, which the final P_ DMA (issued
  last on the same ring) overwrites, and its contribution to the token
  sum is cancelled with a correction DMA of rows x[b+1, 0, :] (zero pad
  row for the last batch).
  Token sums accumulate batch-on-partition in PSUM via indicator-
  stationary matmuls (lhsT = e_b x ones), both u-halves into the same
  bank.  Tiny stage 2/3 computes aq and P_.
Host combines the small pool tensors:
    attnkT[d,k] = att[k,d] * nK[k,d],  attn2T[d,k] = att[k,d]^2,
    prflat[k,:] = prompt[k].reshape(6144)
aq is scale-invariant in x_mean, so the 1/197 mean scaling cancels and
the kernel works with raw token sums.
"""

import numpy as np

TOP_K = 10
LENGTH = 8
EMBED_DIM = 768
N_TOK = 197
B_FULL = 256
N_CORES = 8
B = B_FULL // N_CORES          # 32 batches per core
PF = LENGTH * EMBED_DIM        # 6144 flattened prompt row
XROWS = B * N_TOK + 1          # flat x rows incl one zero pad row
OROWS = B * (LENGTH + N_TOK) + 1   # flat out rows incl pad
NP2 = (N_TOK + 1) // 2         # 99 token pairs per batch (last half garbage)

_PROGRAMS = {}


def _build_program(K):
    import concourse.bacc as bacc
    import concourse.mybir as mybir
    import concourse.tile as tile
    from concourse.bass import ts
    from concourse.masks import make_identity

    f32 = mybir.dt.float32
    nc = bacc.Bacc()

    x = nc.dram_tensor("x", [XROWS, EMBED_DIM], f32, kind="ExternalInput")
    prflat = nc.dram_tensor("prflat", [K, PF], f32, kind="ExternalInput")
    attnkT = nc.dram_tensor("attnkT", [EMBED_DIM, K], f32, kind="ExternalInput")
    attn2T = nc.dram_tensor("attn2T", [EMBED_DIM, K], f32, kind="ExternalInput")
    emat = nc.dram_tensor("emat", [128, B, B], f32, kind="ExternalInput")
    out = nc.dram_tensor("out", [OROWS, EMBED_DIM], f32, kind="ExternalOutput")

    with tile.TileContext(nc) as tc:
        with (
            tc.tile_pool(name="const", bufs=1) as constp,
            tc.tile_pool(name="xt", bufs=8) as xtp,
            tc.tile_pool(name="xs", bufs=6) as xsp,
            tc.tile_pool(name="misc", bufs=1) as miscp,
            tc.tile_pool(name="psA", bufs=1, space="PSUM") as psap,
            tc.tile_pool(name="pst", bufs=1, space="PSUM") as pstp,
            tc.tile_pool(name="pp", bufs=2, space="PSUM") as ppp,
            tc.tile_pool(name="pt", bufs=2, space="PSUM") as ptp,
        ):
            # --- constants (gpsimd queue; big streams go on sync/scalar) ---
            ident = constp.tile([128, 128], f32)
            make_identity(nc, ident)
            prflat_sb = constp.tile([K, PF], f32)
            nc.gpsimd.dma_start(out=prflat_sb, in_=prflat[:, :])
            attnkT_sb = constp.tile([128, 6, K], f32)
            nc.gpsimd.dma_start(
                out=attnkT_sb,
                in_=attnkT[:, :].rearrange("(c p) k -> p c k", p=128))
            attn2T_sb = constp.tile([128, 6, K], f32)
            nc.gpsimd.dma_start(
                out=attn2T_sb,
                in_=attn2T[:, :].rearrange("(c p) k -> p c k", p=128))
            emat_sb = constp.tile([128, B, B], f32)
            nc.gpsimd.dma_start(out=emat_sb, in_=emat[:, :, :])
            # correction rows: x[b+1, token 0] for each b (pad row = 0 last)
            corr_sb = constp.tile([B, EMBED_DIM], f32)
            import concourse.bass as bass
            corr_ap = bass.AP(tensor=x[:, :].tensor, offset=N_TOK * EMBED_DIM,
                              ap=[[N_TOK * EMBED_DIM, B], [1, EMBED_DIM]])
            nc.gpsimd.dma_start(out=corr_sb, in_=corr_ap)

            # Preheat: have PE consume each constant once so no later matmul
            # needs >1 semaphore wait.
            scr = ptp.tile([1, 1], f32, tag="pt", name="scr")
            nc.tensor.matmul(scr, ident[:1, :1], ident[:1, :1],
                             start=True, stop=True)
            nc.tensor.matmul(scr, attnkT_sb[:1, 0, :1], attnkT_sb[:1, 0, :1],
                             start=True, stop=True)
            nc.tensor.matmul(scr, attn2T_sb[:1, 0, :1], attn2T_sb[:1, 0, :1],
                             start=True, stop=True)
            nc.tensor.matmul(scr, prflat_sb[:1, :1], prflat_sb[:1, :1],
                             start=True, stop=True)
            nc.tensor.matmul(scr, emat_sb[:1, 0, :1], emat_sb[:1, 0, :1],
                             start=True, stop=True)

            # token sums (+garbage), batch-on-partition, 2 psum halves
            psum_h = [psap.tile([B, 384], f32, tag=f"psum{h}", name=f"psum{h}")
                      for h in range(2)]

            # Byte-balance the three DMA queues (sync/scalar HWDGE ~1.0 rel
            # rate, gpsimd SWDGE ~0.56): in-DMAs on sync with a few spilled
            # to scalar; out-DMAs on scalar with some on gpsimd.
            in_eng = [nc.sync] * B
            for b in range(4, B, 4):
                if sum(1 for e in in_eng if e is nc.scalar) < 7:
                    in_eng[b] = nc.scalar
            out_eng = [nc.scalar] * B
            for b in range(1, B, 2):
                if sum(1 for e in out_eng if e is nc.gpsimd) < 14:
                    out_eng[b] = nc.gpsimd

            # --- stage 1: stream x, copy to out rows, accumulate sums ------
            for b in range(B):
                r0 = b * N_TOK
                o0 = b * (LENGTH + N_TOK) + LENGTH
                xt = xtp.tile([NP2, 2, EMBED_DIM], f32)
                in_eng[b].dma_start(
                    out=xt,
                    in_=x[r0:r0 + 2 * NP2, :].rearrange("(p u) d -> p u d", u=2))
                eng = out_eng[b]
                eng.dma_start(
                    out=out[o0:o0 + 2 * NP2, :].rearrange("(p u) d -> p u d",
                                                          u=2),
                    in_=xt)
                # fold the token pairs on DVE: halves the PE streaming volume
                xs = xsp.tile([NP2, EMBED_DIM], f32)
                nc.vector.tensor_add(xs, xt[:, 0, :], xt[:, 1, :])
                for h in range(2):
                    nc.tensor.matmul(
                        psum_h[h],
                        emat_sb[:NP2, b, :], xs[:, ts(h, 384)],
                        start=(b == 0), stop=(b == B - 1))

            # --- stage 2: subtract garbage, transpose, numer/norm2, aq -----
            means = miscp.tile([B, EMBED_DIM], f32)
            for h in range(2):
                nc.vector.tensor_sub(means[:, ts(h, 384)], psum_h[h],
                                     corr_sb[:, ts(h, 384)])

            meansT = miscp.tile([128, 6, B], f32)
            for j in range(6):
                pt = ptp.tile([128, B], f32)
                nc.tensor.transpose(pt, means[:, ts(j, 128)], ident[:B, :B])
                nc.vector.tensor_copy(meansT[:, j, :], pt)
            sqT = miscp.tile([128, 6, B], f32)
            nc.vector.tensor_mul(sqT, meansT, meansT)

            pn = pstp.tile([K, B], f32)
            pq = pstp.tile([K, B], f32)
            for j in range(6):
                nc.tensor.matmul(pn, attnkT_sb[:, j, :], meansT[:, j, :],
                                 start=(j == 0), stop=(j == 5))
            for j in range(6):
                nc.tensor.matmul(pq, attn2T_sb[:, j, :], sqT[:, j, :],
                                 start=(j == 0), stop=(j == 5))

            denom = miscp.tile([K, B], f32)
            nc.scalar.sqrt(denom, pq)
            nc.vector.tensor_scalar_max(denom, denom, 1e-12)
            recip = miscp.tile([K, B], f32)
            nc.vector.reciprocal(recip, denom)
            aqT = miscp.tile([K, B], f32)
            nc.vector.tensor_mul(aqT, pn, recip)

            # --- stage 3: P_ = aq @ prflat, write out P_ rows last ---------
            p_sb = miscp.tile([B, PF], f32)
            for h in range(PF // 384):
                pp = ppp.tile([B, 384], f32)
                nc.tensor.matmul(pp, aqT, prflat_sb[:, ts(h, 384)],
                                 start=True, stop=True)
                nc.vector.tensor_copy(p_sb[:, ts(h, 384)], pp)
            nc.scalar.dma_start(
                out=out[0:B * (LENGTH + N_TOK), :].rearrange(
                    "(b r) d -> b r d", r=LENGTH + N_TOK)[:, 0:LENGTH, :],
                in_=p_sb.rearrange("p (l d) -> p l d", l=LENGTH))

    nc.finalize()
    return nc


def _host_prep(prompt, attention, prompt_key, task_count):
    K = (int(task_count) + 1) * TOP_K
    pk = np.asarray(prompt_key[:K], dtype=np.float32)
    att = np.asarray(attention[:K], dtype=np.float32)
    pr = np.asarray(prompt[:K], dtype=np.float32)
    nrm = np.sqrt(np.sum(pk * pk, axis=1, keepdims=True, dtype=np.float32))
    nK = pk / np.maximum(nrm, np.float32(1e-12))
    attnkT = np.ascontiguousarray((att * nK).T)
    attn2T = np.ascontiguousarray((att * att).T)
    prflat = np.ascontiguousarray(pr.reshape(K, PF))
    return K, attnkT, attn2T, prflat


def _make_emat():
    emat = np.zeros((128, B, B), dtype=np.float32)
    for b in range(B):
        emat[:, b, b] = 1.0
    return emat


def _shard_x(x_embed, i):
    flat = x_embed[i * B:(i + 1) * B].reshape(B * N_TOK, EMBED_DIM)
    pad = np.zeros((1, EMBED_DIM), dtype=np.float32)
    return np.ascontiguousarray(np.concatenate([flat, pad], axis=0))


def kernel(x_embed, prompt, attention, prompt_key, iseval, task_count,
           _want_trace=False, **_trace_kwargs):
    from concourse.bass_utils import run_bass_kernel_spmd

    x_embed = np.asarray(x_embed, dtype=np.float32)
    assert x_embed.shape == (B_FULL, N_TOK, EMBED_DIM)
    K, attnkT, attn2T, prflat = _host_prep(prompt, attention, prompt_key,
                                           task_count)

    if K not in _PROGRAMS:
        _PROGRAMS[K] = _build_program(K)
    nc = _PROGRAMS[K]

    emat = _make_emat()
    in_maps = []
    for i in range(N_CORES):
        in_maps.append({
            "x": _shard_x(x_embed, i),
            "prflat": prflat,
            "attnkT": attnkT,
            "attn2T": attn2T,
            "emat": emat,
        })
    res = run_bass_kernel_spmd(nc, in_maps, core_ids=list(range(N_CORES)),
                               trace=_want_trace, **_trace_kwargs)
    full = np.concatenate(
        [res.results[i]["out"][:B * (LENGTH + N_TOK)].reshape(
            B, LENGTH + N_TOK, EMBED_DIM) for i in range(N_CORES)],
        axis=0)
    if _want_trace:
        return full, res
    return full

